# revision 3
# baseline (speedup 1.0000x reference)
import os
import sys

for _p in ("/opt/trn_rl_repo", "/root/.axon_site/_ro/trn_rl_repo"):
    if os.path.isdir(_p) and _p not in sys.path:
        sys.path.insert(0, _p)

import numpy as np
import concourse.bacc as bacc
import concourse.mybir as mybir
import concourse.tile as tile
from concourse import bass_utils

B, N, T, F = 8, 128, 2048, 32
L, H = 5, 64

FP32 = mybir.dt.float32
FP16 = mybir.dt.float16

HALO = 4          # max_lag - 1
CHUNK = 16        # t-steps per output chunk
NCHUNKS = T // CHUNK  # 128
NG = NCHUNKS // 2     # 64 transpose groups (2 chunks each)

# graduated x tiles: small first tiles so the PE can start early
TILE_T = [32, 96, 128] + [256] * 7
TILE_SLICES = [1, 2, 2] + [8] * 7
TILE_START = [sum(TILE_T[:i]) for i in range(len(TILE_T))]
assert sum(TILE_T) == T

Y_CHUNK_FREE = CHUNK * H       # 1024

_CACHE = {}
LAST_RESULTS = None


def _build_nc():
    nc = bacc.Bacc("TRN2", target_bir_lowering=False, debug=False)
    x_d = nc.dram_tensor("x", (N, T * F), FP16, kind="ExternalInput").ap()
    at_d = nc.dram_tensor("at", (N, L * N), FP16, kind="ExternalInput").ap()
    wd_d = nc.dram_tensor("wd", (128, 256), FP16, kind="ExternalInput").ap()
    bvec_d = nc.dram_tensor("bvec", (128, 1), FP32, kind="ExternalInput").ap()
    y_d = nc.dram_tensor("y", (N, T * H), FP16, kind="ExternalOutput").ap()

    if os.environ.get("SIM_NOGELU"):
        gelu = mybir.ActivationFunctionType.Identity
    else:
        gelu = mybir.ActivationFunctionType.Gelu

    # chunk -> tile index
    tile_of_chunk = []
    for ti, (s, sz) in enumerate(zip(TILE_START, TILE_T)):
        tile_of_chunk += [ti] * (sz // CHUNK)
    first_chunk_of_tile = {}
    for g, ti in enumerate(tile_of_chunk):
        first_chunk_of_tile.setdefault(ti, g)

    with tile.TileContext(nc) as tc:
        with (
            tc.tile_pool(name="consts", bufs=1) as consts,
            tc.tile_pool(name="xpool", bufs=2) as xpool,
            tc.tile_pool(name="trpool", bufs=3) as trpool,
            tc.tile_pool(name="tpool", bufs=3) as tpool,
            tc.tile_pool(name="ypool", bufs=16) as ypool,
            tc.tile_pool(name="pagg", bufs=2, space="PSUM") as pagg,
            tc.tile_pool(name="py", bufs=2, space="PSUM") as py,
        ):
            at_sb = consts.tile((N, L * N), FP16)
            wd_sb = consts.tile((128, 256), FP16)
            bvec_sb = consts.tile((128, 1), FP32)
            warm_sb = consts.tile((1, 2), FP32)
            # tiny activation with no DMA deps: pulls the gelu ACT_TABLE_LOAD
            # into the preamble window instead of behind the x DMAs
            nc.any.memset(warm_sb, 0.0)
            nc.scalar.activation(warm_sb, warm_sb, func=gelu)
            # dummy matmuls on zeroed SBUF during the x-DMA wait: accumulate
            # ~3.4us of PE activity so HAM unthrottles before the real stream
            pewarm_sb = consts.tile((N, 128), FP16)
            nc.vector.memset(pewarm_sb, 0.0)
            psum_warm = pagg.tile((N, 1024), FP32, tag="pagg2")
            for _w in range(32):
                nc.tensor.matmul(
                    psum_warm[:, 0:128],
                    pewarm_sb,
                    pewarm_sb,
                    start=True,
                    stop=True,
                )
            nc.sync.dma_start(out=at_sb, in_=at_d)

            x_tiles = {}
            paggs = {}
            trs = {}
            t16s = {}

            def emit_xload(ti):
                sz = TILE_T[ti]
                s = TILE_START[ti]
                nsl = TILE_SLICES[ti]
                free = (sz + HALO) * F
                x_tile = xpool.tile((N, free), FP16, tag="x")
                x_tiles[ti] = x_tile
                if ti == 0:
                    nc.any.memset(x_tile[:, 0 : HALO * F], 0.0)
                    src = x_d[:, 0 : sz * F]
                    sl = sz * F // nsl
                    for q in range(nsl):
                        nc.sync.dma_start(
                            out=x_tile[:, HALO * F + q * sl : HALO * F + (q + 1) * sl],
                            in_=src[:, q * sl : (q + 1) * sl],
                        )
                else:
                    src = x_d[:, (s - HALO) * F : (s + sz) * F]
                    sl = free // nsl
                    assert free % nsl == 0
                    for q in range(nsl):
                        nc.sync.dma_start(
                            out=x_tile[:, q * sl : (q + 1) * sl],
                            in_=src[:, q * sl : (q + 1) * sl],
                        )

            def emit_s1(g):
                # chunk g into half (g % 2) of group j = g // 2
                j, half = divmod(g, 2)
                if half == 0:
                    psum_agg2 = pagg.tile((N, 1024), FP32, tag="pagg2")
                    paggs[j] = psum_agg2
                psum_agg = paggs[j]
                ti = tile_of_chunk[g]
                x_tile = x_tiles[ti]
                t0 = g * CHUNK
                base = (t0 - TILE_START[ti] + HALO) * F
                for lag in range(L):
                    off = base - lag * F
                    nc.tensor.matmul(
                        psum_agg[:, half * 512 : (half + 1) * 512],
                        at_sb[:, lag * N : (lag + 1) * N],
                        x_tile[:, off : off + 512],
                        start=(lag == 0),
                        stop=(lag == L - 1),
                    )

            def emit_tr(j):
                psum_agg = paggs.pop(j)
                sbuf_tr = trpool.tile((N, 1024), FP32)
                nc.vector.transpose(sbuf_tr, psum_agg)
                trs[j] = sbuf_tr

            def emit_cast(j):
                sbuf_tr = trs.pop(j)
                sbuf_t = tpool.tile((N, 1024), FP16)
                nc.vector.tensor_copy(sbuf_t, sbuf_tr)
                t16s[j] = sbuf_t

            def emit_tr_half(g):
                # per-chunk transpose+cast for the last group: lets the DVE
                # start on half 0 while the PE still runs half 1's matmuls
                j, half = divmod(g, 2)
                psum_agg = paggs[j]
                if half == 1:
                    paggs.pop(j)
                sbuf_trh = trpool.tile((N, 512), FP32, tag="trh")
                nc.vector.transpose(sbuf_trh, psum_agg[:, half * 512 : (half + 1) * 512])
                sbuf_th = tpool.tile((N, 512), FP16, tag="th")
                nc.vector.tensor_copy(sbuf_th, sbuf_trh)
                t16s[("h", g)] = sbuf_th

            def emit_s2(g):
                j, half = divmod(g, 2)
                if ("h", g) in t16s:
                    rhs = t16s.pop(("h", g))
                else:
                    sbuf_t = t16s[j]
                    if half == 1:
                        t16s.pop(j)
                    rhs = sbuf_t[:, half * 512 : (half + 1) * 512]
                psum_y = py.tile((N, Y_CHUNK_FREE), FP32)
                for r in range(2):
                    nc.tensor.matmul(
                        psum_y[:, r * 512 : (r + 1) * 512],
                        wd_sb[:, r * 128 : (r + 1) * 128],
                        rhs,
                        start=True,
                        stop=True,
                    )
                sbuf_y = ypool.tile((N, Y_CHUNK_FREE), FP16)
                nc.scalar.activation(sbuf_y, psum_y, func=gelu, bias=bvec_sb)
                nc.sync.dma_start(
                    out=y_d[:, g * Y_CHUNK_FREE : (g + 1) * Y_CHUNK_FREE], in_=sbuf_y
                )

            emit_xload(0)
            emit_xload(1)
            nc.sync.dma_start(out=wd_sb, in_=wd_d)
            nc.sync.dma_start(out=bvec_sb, in_=bvec_d)
            # group pipeline: s1 pair (j) | tr (j-1) | cast (j-1) | s2 pair (j-2)
            for j in range(NG + 2):
                if j < NG:
                    g0 = 2 * j
                    ti = tile_of_chunk[g0]
                    if j >= NG - 2:
                        emit_s1(g0)
                        emit_tr_half(g0)
                        emit_s1(g0 + 1)
                        emit_tr_half(g0 + 1)
                    else:
                        emit_s1(g0)
                        if (
                            first_chunk_of_tile[ti] == g0
                            and ti >= 1
                            and ti + 1 < len(TILE_T)
                        ):
                            emit_xload(ti + 1)
                        emit_s1(g0 + 1)
                if 1 <= j <= NG - 2:
                    emit_tr(j - 1)
                    emit_cast(j - 1)
                if j >= 2:
                    emit_s2(2 * (j - 2))
                    emit_s2(2 * (j - 2) + 1)
    nc.compile()
    return nc


def _host_inputs(x, A_list, W, b):
    # wd holds the two S2 lhsT matrices side by side:
    # lhsT_r[32*g + f, 64*d + h] = W[h, f] if g == 2*r + d else 0
    wd = np.zeros((128, 256), np.float16)
    wt = W.T.astype(np.float16)  # [f, h] = [32, 64]
    for r in range(2):
        for d in range(2):
            g = 2 * r + d
            wd[32 * g : 32 * g + 32, 128 * r + 64 * d : 128 * r + 64 * d + 64] = wt
    bvec = np.ascontiguousarray(np.tile(b, 2)[:, None].astype(np.float32))

    in_maps = []
    for c in range(x.shape[0]):
        in_maps.append(
            {
                "x": x[c].reshape(N, T * F).astype(np.float16),
                "at": np.ascontiguousarray(
                    A_list[c].transpose(2, 0, 1).reshape(N, L * N)
                ).astype(np.float16),
                "wd": wd,
                "bvec": bvec,
            }
        )
    return in_maps


def _decode_y(arr):
    # arr: [128, T*H] partitions p = 64*d + h;
    # free col = g*1024 + r*512 + tl*32 + il;
    # value = z[i = 64*r + 32*d + il, t = 16*g + tl, h]
    arr6 = arr.reshape(2, 64, T // CHUNK, 2, CHUNK, 32)
    yb = (
        np.transpose(arr6, (3, 0, 5, 2, 4, 1))
        .reshape(N, T, H)
        .astype(np.float32)
    )
    return yb


def kernel(x, A_list, W, b):
    global LAST_RESULTS
    x = np.asarray(x, np.float32)
    A_list = np.asarray(A_list, np.float32)
    W = np.asarray(W, np.float32)
    b = np.asarray(b, np.float32)

    if "nc" not in _CACHE:
        _CACHE["nc"] = _build_nc()
    nc = _CACHE["nc"]

    in_maps = _host_inputs(x, A_list, W, b)

    trace = bool(os.environ.get("KERNEL_TRACE"))
    res = bass_utils.run_bass_kernel_spmd(
        nc, in_maps, core_ids=list(range(B)), trace=trace
    )
    LAST_RESULTS = res
    outs = []
    for c in range(x.shape[0]):
        arr = np.asarray(res.results[c]["y"])
        outs.append(_decode_y(arr))
    return np.stack(outs)
